# revision 1
# baseline (speedup 1.0000x reference)
"""Trainium2 Bass kernel for nn_ApplyAttentionPolicyMap.

Reference computes out = concat(logits, pp_logits) @ fc1 where fc1 is a
4288x1858 one-hot column-selection map: out[b, j] = flat[b, sel[j]].

Strategy (8 NeuronCores, data-parallel over batch):
  * Host: shard the batch 8-ways; each core's activation shard is laid out
    feature-major (xT [4288, 1024]) so the selection becomes a row gather.
    fc1 is reduced to its sparse index form sel[1858] (as the sharding hint
    suggests) and replicated to every core as an int16 index tensor.
  * Device: gpsimd dma_gather pulls the 1858 selected feature rows from HBM
    straight into SBUF ([j%128 partition, j//128 chunk, 1024 batch]); the PE
    transposes each [128,128] block back to batch-major via identity matmul;
    DVE/ACT evacuate PSUM into the output staging tile; HWDGE writes the
    final row-major [1024, 1858] shard to DRAM.  Gather/transpose/copy/store
    are pipelined in 5 column groups.
"""

import numpy as np

import concourse.bacc as bacc
import concourse.mybir as mybir
from concourse.bass_utils import run_bass_kernel_spmd

N_CORES = 8
B = 8192
B_SHARD = B // N_CORES            # 1024
IN_DIM = 64 * 64 + 8 * 24         # 4288
OUT_DIM = 1858
N_BTILE = B_SHARD // 128          # 8 batch sub-tiles per core
N_CHUNK = 15                      # ceil(1858/128) output column chunks
NUM_IDX = N_CHUNK * 128           # 1920 padded gather indices
CHUNKS_PER_GROUP = 3              # pipeline granularity
N_GROUP = N_CHUNK // CHUNKS_PER_GROUP
IDX_FREE = NUM_IDX // 16          # 120 int16 per partition

_DT = mybir.dt.float32

_cached = {}


def _build_nc():
    nc = bacc.Bacc("TRN2")
    xT = nc.declare_dram_parameter("xT", [IN_DIM, B_SHARD], _DT, isOutput=False)
    idx_d = nc.declare_dram_parameter("idx", [128, IDX_FREE], mybir.dt.int16, isOutput=False)
    ident_d = nc.declare_dram_parameter("ident", [128, 128], _DT, isOutput=False)
    out_d = nc.declare_dram_parameter("out", [B_SHARD, OUT_DIM], _DT, isOutput=True)

    # DRAM view of out with batch sub-tile explicit: partition = row within
    # sub-tile, free dims = (sub-tile, column).
    out_v = out_d[:, :].rearrange("(t p) n -> p t n", p=128)

    cpg = CHUNKS_PER_GROUP
    idx_cols_per_group = cpg * 128 // 16   # 24

    with (
        nc.sbuf_tensor("gath", [128, N_CHUNK, B_SHARD], _DT) as gath,
        nc.sbuf_tensor("outb", [128, N_BTILE, NUM_IDX], _DT) as outb,
        nc.sbuf_tensor("idx_sb", [128, IDX_FREE], mybir.dt.int16) as idx_sb,
        nc.sbuf_tensor("ident_sb", [128, 128], _DT) as ident_sb,
        nc.psum_tensor("pt", [128, 8, 512], _DT) as pt,
        nc.semaphore("io") as io_sem,
        nc.semaphore("g0") as gs0,
        nc.semaphore("g1") as gs1,
        nc.semaphore("g2") as gs2,
        nc.semaphore("g3") as gs3,
        nc.semaphore("g4") as gs4,
        nc.semaphore("mm") as mm_sem,
        nc.semaphore("dve") as dve_sem,
        nc.semaphore("act") as act_sem,
        nc.semaphore("outs") as out_sem,
        nc.Block() as block,
    ):
        gsem = [gs0, gs1, gs2, gs3, gs4]

        @block.gpsimd
        def _(g):
            g.dma_start(idx_sb[:, :], idx_d[:, :]).then_inc(io_sem, 16)
            g.dma_start(ident_sb[:, :], ident_d[:, :]).then_inc(io_sem, 16)
            g.wait_ge(io_sem, 32)
            for gg in range(N_GROUP):
                c0 = gg * cpg
                n_valid = min(OUT_DIM, (c0 + cpg) * 128) - c0 * 128
                g.dma_gather(
                    gath[:, c0 : c0 + cpg, :],
                    xT[:, :],
                    idx_sb[:, gg * idx_cols_per_group : (gg + 1) * idx_cols_per_group],
                    cpg * 128,
                    n_valid,
                    B_SHARD,
                ).then_inc(gsem[gg], 16)

        @block.tensor
        def _(t):
            t.wait_ge(io_sem, 32)
            for gg in range(N_GROUP):
                t.wait_ge(gsem[gg], 16)
                for ci in range(cpg):
                    c = gg * cpg + ci
                    for bb in range(N_BTILE):
                        k = c * N_BTILE + bb
                        bank = k % 8
                        if k >= 8:
                            m = k - 8
                            if bank < 4:
                                t.wait_ge(dve_sem, (m // 8) * 4 + bank + 1)
                            else:
                                t.wait_ge(act_sem, (m // 8) * 4 + (bank - 4) + 1)
                        t.matmul(
                            pt[:, bank, 0:128],
                            gath[:, c, bb * 128 : (bb + 1) * 128],
                            ident_sb[:, :],
                            is_transpose=True,
                            start=True,
                            stop=True,
                        ).then_inc(mm_sem, 1)

        @block.vector
        def _(v):
            for m in range(N_CHUNK * N_BTILE):
                if m % 8 >= 4:
                    continue
                c, bb = m // 8, m % 8
                v.wait_ge(mm_sem, m + 1)
                v.tensor_copy(
                    out=outb[:, bb, c * 128 : (c + 1) * 128],
                    in_=pt[:, m % 8, 0:128],
                ).then_inc(dve_sem, 1)

        @block.scalar
        def _(s):
            for m in range(N_CHUNK * N_BTILE):
                if m % 8 < 4:
                    continue
                c, bb = m // 8, m % 8
                s.wait_ge(mm_sem, m + 1)
                s.copy(
                    out=outb[:, bb, c * 128 : (c + 1) * 128],
                    in_=pt[:, m % 8, 0:128],
                ).then_inc(act_sem, 1)

        @block.sync
        def _(s):
            copies_per_group = cpg * N_BTILE // 2  # 12 per engine
            for gg in range(N_GROUP):
                s.wait_ge(dve_sem, copies_per_group * (gg + 1))
                s.wait_ge(act_sem, copies_per_group * (gg + 1))
                col0 = gg * cpg * 128
                col1 = min(col0 + cpg * 128, OUT_DIM)
                s.dma_start(
                    out=out_v[:, :, col0:col1],
                    in_=outb[:, :, col0:col1],
                ).then_inc(out_sem, 16)
            s.wait_ge(out_sem, 16 * N_GROUP)

        # Leave all kernel semaphores at 0 so a re-execution of this NEFF
        # starts from clean state.
        nc.all_engine_barrier()
        for sem in [io_sem, *gsem, mm_sem, dve_sem, act_sem, out_sem]:
            nc.gpsimd.sem_clear(sem)

    nc.compile()
    return nc


def _get_nc():
    if "nc" not in _cached:
        _cached["nc"] = _build_nc()
    return _cached["nc"]


def _extract_sel(fc1: np.ndarray):
    """Return sel[j] with fc1 == one_hot(sel), or None if fc1 is not an
    exact one-hot column-selection map."""
    if fc1.shape != (IN_DIM, OUT_DIM):
        return None
    sel = np.argmax(fc1, axis=0)
    ok = (fc1[sel, np.arange(OUT_DIM)] == 1.0).all()
    if not ok:
        return None
    # each column must have exactly one nonzero
    nnz = np.count_nonzero(fc1, axis=0)
    if not (nnz == 1).all():
        return None
    return sel.astype(np.int64)


def _build_idx_tensor(sel: np.ndarray) -> np.ndarray:
    """int16 [128, IDX_FREE], index j stored at partition j%16 (replicated
    across the eight 16-partition groups), free slot j//16; padding = -1."""
    sel_pad = np.full(NUM_IDX, -1, dtype=np.int16)
    sel_pad[:OUT_DIM] = sel.astype(np.int16)
    wrapped = sel_pad.reshape(IDX_FREE, 16).T  # [16, IDX_FREE]
    return np.tile(wrapped, (8, 1)).copy()  # [128, IDX_FREE]


def kernel(logits: np.ndarray, pp_logits: np.ndarray, fc1: np.ndarray) -> np.ndarray:
    logits = np.asarray(logits, dtype=np.float32)
    pp_logits = np.asarray(pp_logits, dtype=np.float32)
    fc1 = np.asarray(fc1, dtype=np.float32)
    b = logits.shape[0]
    flat = np.concatenate(
        [logits.reshape(b, 64 * 64), pp_logits.reshape(b, 8 * 24)], axis=1
    )

    sel = _extract_sel(fc1)
    if sel is None or b != B:
        # Degenerate input (fc1 not an exact selection map, or unexpected
        # batch) — fall back to the dense reference computation.
        return flat @ fc1

    nc = _get_nc()
    idx_np = _build_idx_tensor(sel)
    ident_np = np.eye(128, dtype=np.float32)
    xT = np.ascontiguousarray(flat.T)  # [4288, 8192]

    in_maps = []
    for i in range(N_CORES):
        shard = np.ascontiguousarray(xT[:, i * B_SHARD : (i + 1) * B_SHARD])
        in_maps.append({"xT": shard, "idx": idx_np, "ident": ident_np})

    res = run_bass_kernel_spmd(nc, in_maps, list(range(N_CORES)))
    out = np.concatenate([res.results[i]["out"] for i in range(N_CORES)], axis=0)
    return np.ascontiguousarray(out.astype(np.float32))
